# revision 53
# baseline (speedup 1.0000x reference)
"""Trainium2 Bass kernel for the gated-attention nn.Module (v29).

Math (per batch element b):
    deg   = rel_pos.sum(-1)                        # [N]
    gate  = sigmoid(deg * W_d + b_d)               # [N, D]
    xg    = x * gate
    qkv   = xg @ W_qkv.T + b_qkv                   # [N, 3D]
    qk, value, res = split(qkv); qk = sigmoid(qk)
    attn  = (qk @ qk.T) * scale * rel_pos          # [N, N]
    attn  = attn / (attn.sum(-1, keepdims) + 1e-6)
    out   = relu(attn @ value + res)               # [N, D]

Sharding: pure data-parallel over batch, B == 8 == n_cores, one batch
element per NeuronCore, no collectives.

v29 design (~133 us, vs 158-160 us for the v20 baseline):
  * rel_pos is staged in HBM as bf16 (the kernel's internal precision
    for it anyway) and read twice -- once naturally for the deg/gate/
    qkv pipeline, once transposed via XBAR directly from DRAM.  Total
    HBM traffic is identical to one f32 read, but the transpose no
    longer serializes against an SBUF source stream: phase A is a pure
    natural stream, phase B is a pure XBAR stream (one mode switch).
    A strict barrier separates the two DMA phases (DMA-transpose is
    mutually exclusive with all other DMA, and data deps cannot hold
    XBARs back); all of phase A's compute is emitted after the barrier
    so it overlaps the XBAR stream.
  * deg is one DVE op per row-tile: fold the two halves with op1=add
    and take the row sum via accum_out in the same instruction, so the
    nat-pool recycling (which paces the stream) is never compute-bound.
  * attn^T is computed in place over the transposed rel (S = qk@qk.T is
    symmetric, so natural-orientation score blocks times relT give
    attn^T directly).  No post-hoc transpose of attn.
  * fp8e4 DoubleRow matmuls for the scores (full 256-contraction per
    instruction).  attn rows split per pair: even pairs quantize to
    fp8 (direct PSUM multiply) and use DoubleRow in attn@value; odd
    pairs stay bf16 in-place (2x DVE multiply via an ACT pre-copy).
    Both accumulate into the same PSUM tile.
  * SCALE dropped (cancels in the normalization; eps rescaled).
  * row-sum normalization via a 257th all-ones column of value.
"""

import math
from contextlib import ExitStack

import numpy as np
import ml_dtypes

import concourse.bass as bass
import concourse.tile as tile
from concourse import bacc, mybir
from concourse.bass import ts
from concourse.bass_utils import run_bass_kernel_spmd
from concourse.masks import make_identity

B, N, D = 8, 2048, 256
E = 3 * D  # 768
NT = N // 128  # 16 row tiles
NP = NT // 2  # 8 tile pairs
SCALE = 1.0 / math.sqrt(32.0)
EPS = 1e-6 / SCALE  # eps rescaled because SCALE is folded out

F32 = mybir.dt.float32
BF16 = mybir.dt.bfloat16
FP8 = mybir.dt.float8e4

AL = mybir.AluOpType
AF = mybir.ActivationFunctionType
DR = mybir.MatmulPerfMode.DoubleRow


def build_kernel(ctx: ExitStack, tc: tile.TileContext, io: dict):
    nc = tc.nc
    x_d = io["x"]          # [N, D]   bf16 (host-staged)
    rel_d = io["rel_pos"]  # [N, N]   bf16 (host-staged)
    wq_d = io["W_qkv"]     # [E, D]   f32
    bq_d = io["b_qkv"]     # [E]      f32
    wd_d = io["W_d"]       # [D, 1]   f32
    bd_d = io["b_d"]       # [D]      f32
    out_d = io["out"]      # [N, D]   f32

    # ---------------- pools ----------------
    consts = ctx.enter_context(tc.tile_pool(name="consts", bufs=1))
    resid = ctx.enter_context(tc.tile_pool(name="resid", bufs=1))
    natp = ctx.enter_context(tc.tile_pool(name="natp", bufs=4))
    xpool = ctx.enter_context(tc.tile_pool(name="xpool", bufs=4))
    wk = ctx.enter_context(tc.tile_pool(name="wk", bufs=2))
    small = ctx.enter_context(tc.tile_pool(name="small", bufs=4))
    ps = ctx.enter_context(tc.tile_pool(name="ps", bufs=2, space="PSUM"))
    pap = ctx.enter_context(tc.tile_pool(name="pap", bufs=4, space="PSUM"))
    pso = ctx.enter_context(tc.tile_pool(name="pso", bufs=2, space="PSUM"))

    # ---------------- resident tensors ----------------
    # rbT[p, i, j, q] = rel[128i+q, 128j+p]: XBAR of DRAM row-block i writes
    # the contiguous slab rbT[:, i].  Odd row-pairs are overwritten in place
    # with bf16 attn^T by the multiply.
    rbT = resid.tile([128, NT, NT, 128], BF16)
    # rb8[p, e, t, n] = attn_un^T[128(4e+t)+p, n] for even pairs (fp8)
    rb8 = resid.tile([128, 4, 2, N], FP8)
    # qk8[p, s, n] = sigmoid(qk)[n, 128s+p]  (fp8, DoubleRow layout)
    qk8 = resid.tile([128, 2, N], FP8)
    # xgT[p, dc, n] = xg[n, 128dc+p]  (bf16)
    xgT = resid.tile([128, 2, N], BF16)
    # gated x rows, produced pre-barrier (gates need only deg + consts)
    xg_all = resid.tile([128, NT, D], BF16)
    # value rows + bias; d=256 col is all-ones.  fp8 copy for even pairs,
    # bf16 copy for odd pairs.
    vpx8 = resid.tile([128, NT, 264], FP8)
    vpxb = resid.tile([128, NT, 264], BF16)
    res = resid.tile([128, NT, D], BF16)
    deg = resid.tile([128, NT], F32)
    dscr = resid.tile([128, N // 2], BF16)
    o_all = resid.tile([128, NT, D], BF16)

    # ---------------- constants ----------------
    ident = consts.tile([128, 128], BF16)
    make_identity(nc, ident)

    wd_bc = consts.tile([128, D], F32)
    nc.scalar.dma_start(
        out=wd_bc,
        in_=bass.AP(tensor=wd_d.tensor, offset=wd_d.offset, ap=[[0, 128], [1, D]]),
    )
    bd_bc = consts.tile([128, D], F32)
    nc.scalar.dma_start(
        out=bd_bc,
        in_=bass.AP(tensor=bd_d.tensor, offset=bd_d.offset, ap=[[0, 128], [1, D]]),
    )
    # per-partition qk bias: bqv[p, ec] = b_qkv[128*ec + p]
    bqv = consts.tile([128, 2], F32)
    nc.scalar.dma_start(
        out=bqv,
        in_=bass.AP(tensor=bq_d.tensor, offset=bq_d.offset, ap=[[1, 128], [128, 2]]),
    )
    ones_row = consts.tile([1, 512], BF16)
    nc.vector.memset(ones_row, 1.0)
    bq_row_f = consts.tile([1, E], F32)
    nc.scalar.dma_start(
        out=bq_row_f,
        in_=bass.AP(tensor=bq_d.tensor, offset=bq_d.offset, ap=[[1, 1], [1, E]]),
    )
    bq_row = consts.tile([1, E], BF16)
    nc.vector.tensor_copy(out=bq_row, in_=bq_row_f)

    nc.vector.memset(vpx8[:, :, 256:257], 1.0)
    nc.vector.memset(vpxb[:, :, 256:257], 1.0)

    # ---------------- phase A: natural stream ----------------
    rel_src = rel_d.rearrange("(c h p) k -> c p h k", p=128, h=2)
    nat = [natp.tile([128, 2, N], BF16, tag="nat", name=f"nat{c}") for c in range(NP - 1)]
    nat.append(resid.tile([128, 2, N], BF16, name="nat_last"))  # resident: clean RAW gate
    nc.scalar.dma_start(out=nat[0], in_=rel_src[0])
    wq_nat, free_wq_nat = tc.tile([128, 6, D], BF16, name="wq_nat")
    nc.gpsimd.dma_start(out=wq_nat, in_=wq_d.rearrange("(c p) d -> p c d", p=128))
    xt4 = [xpool.tile([128, 4, D], BF16, tag="xt4", name=f"xt4_{g}") for g in range(4)]
    for g in range(4):
        nc.gpsimd.dma_start(
            out=xt4[g], in_=x_d.rearrange("(g q p) d -> g p q d", p=128, q=4)[g]
        )
    for c in range(1, NP):
        nc.scalar.dma_start(out=nat[c], in_=rel_src[c])

    def ea_deg(i):
        # one DVE op: dscr = lo + hi halves (2x bf16), accum_out = full row sum
        nc.vector.scalar_tensor_tensor(
            out=dscr[:, 0 : N // 2], in0=nat[i // 2][:, i % 2, 0 : N // 2],
            scalar=1.0, in1=nat[i // 2][:, i % 2, N // 2 : N],
            op0=AL.mult, op1=AL.add, accum_out=deg[:, i : i + 1],
        )

    def ea_gate(i):
        gf = wk.tile([128, D], BF16, tag="gf", name="gf")
        nc.vector.scalar_tensor_tensor(
            out=gf, in0=wd_bc, scalar=deg[:, i : i + 1], in1=bd_bc,
            op0=AL.mult, op1=AL.add,
        )
        gate = wk.tile([128, D], BF16, tag="gate", name="gate")
        nc.scalar.activation(out=gate, in_=gf, func=AF.Sigmoid)
        nc.vector.tensor_tensor(
            out=xg_all[:, i, :], in0=xt4[i // 4][:, i % 4, :], in1=gate, op=AL.mult
        )

    def ea_xgT(h, xga, xgb):
        pt = ps.tile([128, 512], BF16, tag="ps", name="pt_xg", padded_shape=[128, 1024])
        for t, xg in enumerate((xga, xgb)):
            for dc in range(2):
                nc.tensor.transpose(
                    pt[:, dc * 256 + t * 128 : dc * 256 + (t + 1) * 128],
                    xg[:, ts(dc, 128)],
                    ident,
                )
        for dc in range(2):
            nc.vector.tensor_copy(
                out=xgT[:, dc, ts(h, 256)], in_=pt[:, ts(dc, 256)]
            )

    def ea_vr(i):
        pv = ps.tile([128, 512], F32, tag="ps", name="pv")
        for dc in range(2):
            nc.tensor.matmul(
                pv, lhsT=xgT[:, dc, ts(i, 128)], rhs=wqT[dc][:, D : 3 * D],
                start=(dc == 0), stop=False,
            )
        nc.tensor.matmul(
            pv, lhsT=ones_row[:, 0:128], rhs=bq_row[:, D : 3 * D],
            start=False, stop=True,
        )
        if (i // 2) % 2 == 0:
            nc.scalar.copy(out=vpx8[:, i, 0:256], in_=pv[:, 0:D])
        else:
            nc.scalar.copy(out=vpxb[:, i, 0:256], in_=pv[:, 0:D])
        nc.vector.tensor_copy(out=res[:, i, :], in_=pv[:, D : 2 * D])

    def ea_qk(h):
        for ec in range(2):
            pq = ps.tile([128, 256], F32, tag="ps", name="pq", padded_shape=[128, 512])
            for dc in range(2):
                nc.tensor.matmul(
                    pq, lhsT=wqT[dc][:, ts(ec, 128)], rhs=xgT[:, dc, ts(h, 256)],
                    start=(dc == 0), stop=(dc == 1),
                )
            nc.scalar.activation(
                out=qk8[:, ec, ts(h, 256)], in_=pq, func=AF.Sigmoid,
                bias=bqv[:, ec : ec + 1],
            )

    # ---------------- W_qkv -> wqT ----------------
    ea_deg(0)
    ea_deg(1)
    wqT = [consts.tile([128, E], BF16, tag=f"wqT{dc}", name=f"wqT{dc}") for dc in range(2)]
    for c in range(6):
        for dc in range(2):
            pt = ps.tile([128, 128], BF16, tag="ps", name="pt_w", padded_shape=[128, 1024])
            nc.tensor.transpose(pt, wq_nat[:, c, ts(dc, 128)], ident)
            nc.vector.tensor_copy(out=wqT[dc][:, ts(c, 128)], in_=pt)
    free_wq_nat()

    # deg for pairs 1-6 gates the nat-pool recycling, so it precedes the
    # barrier; pair 7 is resident (gates nothing) and its deg is emitted
    # inside the compute loop.  Gates for tiles 0-13 (DVE/ACT only, no PE,
    # so no cold-clock tax) ride along two tiles behind deg, leaving a
    # pure PE+ACT qkv ladder after the barrier.
    for i in range(2, 14):
        ea_deg(i)
        ea_gate(i - 2)
    ea_gate(12)
    ea_gate(13)

    # ---------------- phase B: XBARs from DRAM + scores + attn@value ------
    # The XBARs ride the SAME in-order HWDGE queue (scalar) as the natural
    # stream, so queue order forces them after phase A.  (A data dep can't
    # express this: the transpose dest alias defeats WAW tracking, and the
    # scheduler otherwise runs the dependency-free XBARs first, starving
    # the stream -- DMA-transpose excludes all other DMA.)
    def b_xbar(i):
        # rbT[p, i, j, q] = rel[128i+q, 128j+p], straight from DRAM bf16
        nc.sync.dma_start(
            out=rbT[:, i],
            in_=bass.AP(
                tensor=rel_d.tensor, offset=rel_d.offset + i * 128 * N,
                ap=[[N, 128], [1, N]],
            ),
            transpose=True,
        )

    def b1_block(a, q):
        # attn^T tile a, columns 512q..512q+511
        pa = pap.tile([128, 512], F32, tag="pa", name="pa")
        nc.tensor.matmul(
            pa, lhsT=qk8[:, :, ts(a, 128)], rhs=qk8[:, :, ts(q, 512)],
            perf_mode=DR, start=True, stop=True,
        )
        rsl = rbT[:, 4 * q : 4 * q + 4, a, :]
        if (a // 2) % 2 == 0:
            # fp8 flavor: direct 1x multiply from PSUM
            nc.vector.tensor_tensor(
                out=rb8[:, a // 4, a % 2, ts(q, 512)], in0=pa, in1=rsl, op=AL.mult
            )
        else:
            # bf16 flavor: ACT pre-copy, then 2x bf16 multiply in place
            sb = wk.tile([128, 512], BF16, tag="sb", name="sb")
            nc.scalar.copy(out=sb, in_=pa)
            nc.vector.tensor_tensor(out=rsl, in0=sb, in1=rsl, op=AL.mult)

    def b2_tile(n):
        po = pso.tile([128, 257], F32, tag="po", name="po", padded_shape=[128, 512])
        first = True
        for jj in range(NP):
            if jj % 2 == 0:
                nc.tensor.matmul(
                    po,
                    lhsT=rb8[:, jj // 2, :, ts(n, 128)],
                    rhs=vpx8[:, 2 * jj : 2 * jj + 2, 0:257],
                    perf_mode=DR, start=first, stop=False,
                )
                first = False
            else:
                for j in (2 * jj, 2 * jj + 1):
                    nc.tensor.matmul(
                        po, lhsT=rbT[:, n, j, :], rhs=vpxb[:, j, 0:257],
                        start=False, stop=(j == 2 * NP - 1),
                    )
        z = small.tile([128, 1], F32, tag="z", name="z")
        nc.vector.tensor_scalar_add(out=z, in0=po[:, 256:257], scalar1=EPS)
        zi = small.tile([128, 1], F32, tag="zi", name="zi")
        nc.vector.reciprocal(out=zi, in_=z)
        o1 = wk.tile([128, D], BF16, tag="o1", name="o1")
        nc.scalar.activation(out=o1, in_=po[:, 0:D], func=AF.Copy, scale=zi)
        o2 = wk.tile([128, D], BF16, tag="o2", name="o2")
        nc.vector.tensor_tensor(out=o2, in0=o1, in1=res[:, n, :], op=AL.add)
        nc.scalar.activation(out=o_all[:, n, :], in_=o2, func=AF.Relu)

    # Hard phase boundary: data-dep gates do not hold DMA-transposes back
    # (tried: RAW dummy DMAs on the same ring), and without ordering the
    # scheduler runs the dependency-free XBARs first, starving the phase-A
    # stream (DMA-transpose excludes all other DMA).  Phase-B work depends
    # on the full qk anyway, so the barrier is nearly free.
    tc.strict_bb_all_engine_barrier()
    for i in range(NT):
        b_xbar(i)

    # phase-A compute (gate -> xgT -> value/res -> qk): emitted after the
    # barrier so it overlaps the XBAR stream (compute does not conflict
    # with DMA-transpose).  Interleaving the score blocks here was tried
    # and is a net loss: their multiplies wait on XBAR slabs and block the
    # gate chain on the in-order DVE queue.
    for h in range(NP):
        if h == NP - 1:
            ea_deg(14)
            ea_deg(15)
            ea_gate(14)
            ea_gate(15)
        ea_xgT(h, xg_all[:, 2 * h, :], xg_all[:, 2 * h + 1, :])
        ea_vr(2 * h)
        ea_vr(2 * h + 1)
        ea_qk(h)

    def store(g):
        nc.gpsimd.dma_start(
            out=out_d.rearrange("(g t p) d -> g p t d", p=128, t=4)[g],
            in_=o_all[:, ts(g, 4), :],
        )

    for q in range(4):
        for a in range(NT):
            b1_block(a, q)
        if q > 0:
            # b2 wave lags one chunk so the PE overlaps this chunk's DVE/ACT
            # multiplies with the previous chunk's attn@value
            for n in range(4 * (q - 1), 4 * q):
                b2_tile(n)
            store(q - 1)
    for n in range(12, 16):
        b2_tile(n)
    store(3)


_CACHE: dict = {}


def _get_nc():
    if "nc" in _CACHE:
        return _CACHE["nc"], _CACHE["io"]
    nc = bacc.Bacc("TRN2", target_bir_lowering=False, debug=False)
    io = {
        "x": nc.dram_tensor("x", [N, D], BF16, kind="ExternalInput").ap(),
        "rel_pos": nc.dram_tensor("rel_pos", [N, N], BF16, kind="ExternalInput").ap(),
        "W_qkv": nc.dram_tensor("W_qkv", [E, D], F32, kind="ExternalInput").ap(),
        "b_qkv": nc.dram_tensor("b_qkv", [E], F32, kind="ExternalInput").ap(),
        "W_d": nc.dram_tensor("W_d", [D, 1], F32, kind="ExternalInput").ap(),
        "b_d": nc.dram_tensor("b_d", [D], F32, kind="ExternalInput").ap(),
        "out": nc.dram_tensor("out", [N, D], F32, kind="ExternalOutput").ap(),
    }
    with tile.TileContext(nc) as tc:
        with ExitStack() as ctx:
            build_kernel(ctx, tc, io)
    nc.compile()
    _CACHE["nc"] = nc
    _CACHE["io"] = io
    return nc, io


def kernel(x, rel_pos, W_qkv, b_qkv, W_d, b_d, **run_kwargs):
    nc, _ = _get_nc()
    x = np.ascontiguousarray(np.asarray(x, dtype=np.float32).astype(ml_dtypes.bfloat16))
    rel_bf = np.ascontiguousarray(
        np.asarray(rel_pos, dtype=np.float32).astype(ml_dtypes.bfloat16)
    )
    W_qkv = np.ascontiguousarray(np.asarray(W_qkv, dtype=np.float32))
    b_qkv = np.ascontiguousarray(np.asarray(b_qkv, dtype=np.float32))
    W_d = np.ascontiguousarray(np.asarray(W_d, dtype=np.float32))
    b_d = np.ascontiguousarray(np.asarray(b_d, dtype=np.float32))
    in_maps = [
        {
            "x": x[b],
            "rel_pos": rel_bf[b],
            "W_qkv": W_qkv,
            "b_qkv": b_qkv,
            "W_d": W_d,
            "b_d": b_d,
        }
        for b in range(B)
    ]
    r = run_bass_kernel_spmd(nc, in_maps, core_ids=list(range(B)), **run_kwargs)
    out = np.stack([r.results[b]["out"] for b in range(B)], axis=0)
    if run_kwargs:
        _CACHE["last_result"] = r
    return out


# revision 54
# speedup vs baseline: 1.0697x; 1.0697x over previous
"""Trainium2 Bass kernel for the gated-attention nn.Module (v29).

Math (per batch element b):
    deg   = rel_pos.sum(-1)                        # [N]
    gate  = sigmoid(deg * W_d + b_d)               # [N, D]
    xg    = x * gate
    qkv   = xg @ W_qkv.T + b_qkv                   # [N, 3D]
    qk, value, res = split(qkv); qk = sigmoid(qk)
    attn  = (qk @ qk.T) * scale * rel_pos          # [N, N]
    attn  = attn / (attn.sum(-1, keepdims) + 1e-6)
    out   = relu(attn @ value + res)               # [N, D]

Sharding: pure data-parallel over batch, B == 8 == n_cores, one batch
element per NeuronCore, no collectives.

v29 design (~133 us, vs 158-160 us for the v20 baseline):
  * rel_pos is staged in HBM as bf16 (the kernel's internal precision
    for it anyway) and read twice -- once naturally for the deg/gate/
    qkv pipeline, once transposed via XBAR directly from DRAM.  Total
    HBM traffic is identical to one f32 read, but the transpose no
    longer serializes against an SBUF source stream: phase A is a pure
    natural stream, phase B is a pure XBAR stream (one mode switch).
    A strict barrier separates the two DMA phases (DMA-transpose is
    mutually exclusive with all other DMA, and data deps cannot hold
    XBARs back); all of phase A's compute is emitted after the barrier
    so it overlaps the XBAR stream.
  * deg is one DVE op per row-tile: fold the two halves with op1=add
    and take the row sum via accum_out in the same instruction, so the
    nat-pool recycling (which paces the stream) is never compute-bound.
  * attn^T is computed in place over the transposed rel (S = qk@qk.T is
    symmetric, so natural-orientation score blocks times relT give
    attn^T directly).  No post-hoc transpose of attn.
  * fp8e4 DoubleRow matmuls for the scores (full 256-contraction per
    instruction).  attn rows split per pair: even pairs quantize to
    fp8 (direct PSUM multiply) and use DoubleRow in attn@value; odd
    pairs stay bf16 in-place (2x DVE multiply via an ACT pre-copy).
    Both accumulate into the same PSUM tile.
  * SCALE dropped (cancels in the normalization; eps rescaled).
  * row-sum normalization via a 257th all-ones column of value.
"""

import math
from contextlib import ExitStack

import numpy as np
import ml_dtypes

import concourse.bass as bass
import concourse.tile as tile
from concourse import bacc, mybir
from concourse.bass import ts
from concourse.bass_utils import run_bass_kernel_spmd
from concourse.masks import make_identity

B, N, D = 8, 2048, 256
E = 3 * D  # 768
NT = N // 128  # 16 row tiles
NP = NT // 2  # 8 tile pairs
SCALE = 1.0 / math.sqrt(32.0)
EPS = 1e-6 / SCALE  # eps rescaled because SCALE is folded out

F32 = mybir.dt.float32
BF16 = mybir.dt.bfloat16
FP8 = mybir.dt.float8e4

AL = mybir.AluOpType
AF = mybir.ActivationFunctionType
DR = mybir.MatmulPerfMode.DoubleRow


def build_kernel(ctx: ExitStack, tc: tile.TileContext, io: dict):
    nc = tc.nc
    x_d = io["x"]          # [N, D]   bf16 (host-staged)
    rel_d = io["rel_pos"]  # [N, N]   bf16 (host-staged)
    wq_d = io["W_qkv"]     # [E, D]   f32
    bq_d = io["b_qkv"]     # [E]      f32
    wd_d = io["W_d"]       # [D, 1]   f32
    bd_d = io["b_d"]       # [D]      f32
    out_d = io["out"]      # [N, D]   f32

    # ---------------- pools ----------------
    consts = ctx.enter_context(tc.tile_pool(name="consts", bufs=1))
    resid = ctx.enter_context(tc.tile_pool(name="resid", bufs=1))
    natp = ctx.enter_context(tc.tile_pool(name="natp", bufs=5))
    xpool = ctx.enter_context(tc.tile_pool(name="xpool", bufs=4))
    wk = ctx.enter_context(tc.tile_pool(name="wk", bufs=2))
    small = ctx.enter_context(tc.tile_pool(name="small", bufs=4))
    ps = ctx.enter_context(tc.tile_pool(name="ps", bufs=2, space="PSUM"))
    pap = ctx.enter_context(tc.tile_pool(name="pap", bufs=4, space="PSUM"))
    pso = ctx.enter_context(tc.tile_pool(name="pso", bufs=2, space="PSUM"))

    # ---------------- resident tensors ----------------
    # rbT[p, i, j, q] = rel[128i+q, 128j+p]: XBAR of DRAM row-block i writes
    # the contiguous slab rbT[:, i].  Odd row-pairs are overwritten in place
    # with bf16 attn^T by the multiply.
    rbT = resid.tile([128, NT, NT, 128], BF16)
    # rb8[p, e, t, n] = attn_un^T[128(4e+t)+p, n] for even pairs (fp8)
    rb8 = resid.tile([128, 4, 2, N], FP8)
    # qk8[p, s, n] = sigmoid(qk)[n, 128s+p]  (fp8, DoubleRow layout)
    qk8 = resid.tile([128, 2, N], FP8)
    # xgT[p, dc, n] = xg[n, 128dc+p]  (bf16)
    xgT = resid.tile([128, 2, N], BF16)
    # value rows + bias; d=256 col is all-ones.  fp8 copy for even pairs,
    # bf16 copy for odd pairs.
    vpx8 = resid.tile([128, NT, 264], FP8)
    vpxb = resid.tile([128, NT, 264], BF16)
    res = resid.tile([128, NT, D], BF16)
    deg = resid.tile([128, NT], F32)
    dscr = resid.tile([128, N // 2], BF16)
    o_all = resid.tile([128, NT, D], BF16)

    # ---------------- constants ----------------
    ident = consts.tile([128, 128], BF16)
    make_identity(nc, ident)

    wd_bc = consts.tile([128, D], F32)
    nc.scalar.dma_start(
        out=wd_bc,
        in_=bass.AP(tensor=wd_d.tensor, offset=wd_d.offset, ap=[[0, 128], [1, D]]),
    )
    bd_bc = consts.tile([128, D], F32)
    nc.scalar.dma_start(
        out=bd_bc,
        in_=bass.AP(tensor=bd_d.tensor, offset=bd_d.offset, ap=[[0, 128], [1, D]]),
    )
    # per-partition qk bias: bqv[p, ec] = b_qkv[128*ec + p]
    bqv = consts.tile([128, 2], F32)
    nc.scalar.dma_start(
        out=bqv,
        in_=bass.AP(tensor=bq_d.tensor, offset=bq_d.offset, ap=[[1, 128], [128, 2]]),
    )
    ones_row = consts.tile([1, 512], BF16)
    nc.vector.memset(ones_row, 1.0)
    bq_row_f = consts.tile([1, E], F32)
    nc.scalar.dma_start(
        out=bq_row_f,
        in_=bass.AP(tensor=bq_d.tensor, offset=bq_d.offset, ap=[[1, 1], [1, E]]),
    )
    bq_row = consts.tile([1, E], BF16)
    nc.vector.tensor_copy(out=bq_row, in_=bq_row_f)

    nc.vector.memset(vpx8[:, :, 256:257], 1.0)
    nc.vector.memset(vpxb[:, :, 256:257], 1.0)

    # ---------------- phase A: natural stream ----------------
    rel_src = rel_d.rearrange("(c h p) k -> c p h k", p=128, h=2)
    nat = [natp.tile([128, 2, N], BF16, tag="nat", name=f"nat{c}") for c in range(NP - 1)]
    nat.append(resid.tile([128, 2, N], BF16, name="nat_last"))  # resident: clean RAW gate
    nc.scalar.dma_start(out=nat[0], in_=rel_src[0])
    wq_nat, free_wq_nat = tc.tile([128, 6, D], BF16, name="wq_nat")
    nc.gpsimd.dma_start(out=wq_nat, in_=wq_d.rearrange("(c p) d -> p c d", p=128))
    xt4 = [xpool.tile([128, 4, D], BF16, tag="xt4", name=f"xt4_{g}") for g in range(4)]
    for g in range(4):
        nc.gpsimd.dma_start(
            out=xt4[g], in_=x_d.rearrange("(g q p) d -> g p q d", p=128, q=4)[g]
        )
    for c in range(1, NP):
        nc.scalar.dma_start(out=nat[c], in_=rel_src[c])

    def ea_deg(i):
        # one DVE op: dscr = lo + hi halves (2x bf16), accum_out = full row sum
        nc.vector.scalar_tensor_tensor(
            out=dscr[:, 0 : N // 2], in0=nat[i // 2][:, i % 2, 0 : N // 2],
            scalar=1.0, in1=nat[i // 2][:, i % 2, N // 2 : N],
            op0=AL.mult, op1=AL.add, accum_out=deg[:, i : i + 1],
        )

    def ea_gate(i):
        gf = wk.tile([128, D], BF16, tag="gf", name="gf")
        nc.vector.scalar_tensor_tensor(
            out=gf, in0=wd_bc, scalar=deg[:, i : i + 1], in1=bd_bc,
            op0=AL.mult, op1=AL.add,
        )
        gate = wk.tile([128, D], BF16, tag="gate", name="gate")
        nc.scalar.activation(out=gate, in_=gf, func=AF.Sigmoid)
        xg = wk.tile([128, D], BF16, tag="xg", name="xg")
        nc.vector.tensor_tensor(
            out=xg, in0=xt4[i // 4][:, i % 4, :], in1=gate, op=AL.mult
        )
        return xg

    def ea_xgT(h, xga, xgb):
        pt = ps.tile([128, 512], BF16, tag="ps", name="pt_xg", padded_shape=[128, 1024])
        for t, xg in enumerate((xga, xgb)):
            for dc in range(2):
                nc.tensor.transpose(
                    pt[:, dc * 256 + t * 128 : dc * 256 + (t + 1) * 128],
                    xg[:, ts(dc, 128)],
                    ident,
                )
        for dc in range(2):
            nc.vector.tensor_copy(
                out=xgT[:, dc, ts(h, 256)], in_=pt[:, ts(dc, 256)]
            )

    def ea_vr(i):
        pv = ps.tile([128, 512], F32, tag="ps", name="pv")
        for dc in range(2):
            nc.tensor.matmul(
                pv, lhsT=xgT[:, dc, ts(i, 128)], rhs=wqT[dc][:, D : 3 * D],
                start=(dc == 0), stop=False,
            )
        nc.tensor.matmul(
            pv, lhsT=ones_row[:, 0:128], rhs=bq_row[:, D : 3 * D],
            start=False, stop=True,
        )
        if (i // 2) % 2 == 0:
            nc.scalar.copy(out=vpx8[:, i, 0:256], in_=pv[:, 0:D])
        else:
            nc.scalar.copy(out=vpxb[:, i, 0:256], in_=pv[:, 0:D])
        nc.vector.tensor_copy(out=res[:, i, :], in_=pv[:, D : 2 * D])

    def ea_qk(h):
        for ec in range(2):
            pq = ps.tile([128, 256], F32, tag="ps", name="pq", padded_shape=[128, 512])
            for dc in range(2):
                nc.tensor.matmul(
                    pq, lhsT=wqT[dc][:, ts(ec, 128)], rhs=xgT[:, dc, ts(h, 256)],
                    start=(dc == 0), stop=(dc == 1),
                )
            nc.scalar.activation(
                out=qk8[:, ec, ts(h, 256)], in_=pq, func=AF.Sigmoid,
                bias=bqv[:, ec : ec + 1],
            )

    # ---------------- W_qkv -> wqT ----------------
    ea_deg(0)
    ea_deg(1)
    wqT = [consts.tile([128, E], BF16, tag=f"wqT{dc}", name=f"wqT{dc}") for dc in range(2)]
    for c in range(6):
        for dc in range(2):
            pt = ps.tile([128, 128], BF16, tag="ps", name="pt_w", padded_shape=[128, 1024])
            nc.tensor.transpose(pt, wq_nat[:, c, ts(dc, 128)], ident)
            nc.vector.tensor_copy(out=wqT[dc][:, ts(c, 128)], in_=pt)
    free_wq_nat()

    # deg for pairs 1-6 gates the nat-pool recycling, so it precedes the
    # barrier; pair 7 is resident (gates nothing) and its deg is emitted
    # inside the compute loop, letting the barrier fire at stream end
    # instead of waiting for the last row-sums
    for i in range(2, 14):
        ea_deg(i)

    # ---------------- phase B: XBARs from DRAM + scores + attn@value ------
    # The XBARs ride the SAME in-order HWDGE queue (scalar) as the natural
    # stream, so queue order forces them after phase A.  (A data dep can't
    # express this: the transpose dest alias defeats WAW tracking, and the
    # scheduler otherwise runs the dependency-free XBARs first, starving
    # the stream -- DMA-transpose excludes all other DMA.)
    def b_xbar(i):
        # rbT[p, i, j, q] = rel[128i+q, 128j+p], straight from DRAM bf16
        nc.sync.dma_start(
            out=rbT[:, i],
            in_=bass.AP(
                tensor=rel_d.tensor, offset=rel_d.offset + i * 128 * N,
                ap=[[N, 128], [1, N]],
            ),
            transpose=True,
        )

    def b1_block(a, q):
        # attn^T tile a, columns 512q..512q+511
        pa = pap.tile([128, 512], F32, tag="pa", name="pa")
        nc.tensor.matmul(
            pa, lhsT=qk8[:, :, ts(a, 128)], rhs=qk8[:, :, ts(q, 512)],
            perf_mode=DR, start=True, stop=True,
        )
        rsl = rbT[:, 4 * q : 4 * q + 4, a, :]
        if (a // 2) % 2 == 0:
            # fp8 flavor: direct 1x multiply from PSUM
            nc.vector.tensor_tensor(
                out=rb8[:, a // 4, a % 2, ts(q, 512)], in0=pa, in1=rsl, op=AL.mult
            )
        else:
            # bf16 flavor: ACT pre-copy, then 2x bf16 multiply in place
            sb = wk.tile([128, 512], BF16, tag="sb", name="sb")
            nc.scalar.copy(out=sb, in_=pa)
            nc.vector.tensor_tensor(out=rsl, in0=sb, in1=rsl, op=AL.mult)

    def b2_tile(n):
        po = pso.tile([128, 257], F32, tag="po", name="po", padded_shape=[128, 512])
        first = True
        for jj in range(NP):
            if jj % 2 == 0:
                nc.tensor.matmul(
                    po,
                    lhsT=rb8[:, jj // 2, :, ts(n, 128)],
                    rhs=vpx8[:, 2 * jj : 2 * jj + 2, 0:257],
                    perf_mode=DR, start=first, stop=False,
                )
                first = False
            else:
                for j in (2 * jj, 2 * jj + 1):
                    nc.tensor.matmul(
                        po, lhsT=rbT[:, n, j, :], rhs=vpxb[:, j, 0:257],
                        start=False, stop=(j == 2 * NP - 1),
                    )
        z = small.tile([128, 1], F32, tag="z", name="z")
        nc.vector.tensor_scalar_add(out=z, in0=po[:, 256:257], scalar1=EPS)
        zi = small.tile([128, 1], F32, tag="zi", name="zi")
        nc.vector.reciprocal(out=zi, in_=z)
        o1 = wk.tile([128, D], BF16, tag="o1", name="o1")
        nc.scalar.activation(out=o1, in_=po[:, 0:D], func=AF.Copy, scale=zi)
        o2 = wk.tile([128, D], BF16, tag="o2", name="o2")
        nc.vector.tensor_tensor(out=o2, in0=o1, in1=res[:, n, :], op=AL.add)
        nc.scalar.activation(out=o_all[:, n, :], in_=o2, func=AF.Relu)

    # Hard phase boundary: data-dep gates do not hold DMA-transposes back
    # (tried: RAW dummy DMAs on the same ring), and without ordering the
    # scheduler runs the dependency-free XBARs first, starving the phase-A
    # stream (DMA-transpose excludes all other DMA).  Phase-B work depends
    # on the full qk anyway, so the barrier is nearly free.
    tc.strict_bb_all_engine_barrier()
    for i in range(NT):
        b_xbar(i)

    # phase-A compute (gate -> xgT -> value/res -> qk): emitted after the
    # barrier so it overlaps the XBAR stream (compute does not conflict
    # with DMA-transpose).  Interleaving the score blocks here was tried
    # and is a net loss: their multiplies wait on XBAR slabs and block the
    # gate chain on the in-order DVE queue.
    for h in range(NP):
        if h == NP - 1:
            ea_deg(14)
            ea_deg(15)
        xga = ea_gate(2 * h)
        xgb = ea_gate(2 * h + 1)
        ea_xgT(h, xga, xgb)
        ea_vr(2 * h)
        ea_vr(2 * h + 1)
        ea_qk(h)

    def store(g):
        nc.gpsimd.dma_start(
            out=out_d.rearrange("(g t p) d -> g p t d", p=128, t=4)[g],
            in_=o_all[:, ts(g, 4), :],
        )

    for q in range(4):
        for a in range(NT):
            b1_block(a, q)
        if q > 0:
            # b2 wave lags one chunk so the PE overlaps this chunk's DVE/ACT
            # multiplies with the previous chunk's attn@value
            for n in range(4 * (q - 1), 4 * q):
                b2_tile(n)
            store(q - 1)
    for n in range(12, 16):
        b2_tile(n)
    store(3)


_CACHE: dict = {}


def _get_nc():
    if "nc" in _CACHE:
        return _CACHE["nc"], _CACHE["io"]
    nc = bacc.Bacc("TRN2", target_bir_lowering=False, debug=False)
    io = {
        "x": nc.dram_tensor("x", [N, D], BF16, kind="ExternalInput").ap(),
        "rel_pos": nc.dram_tensor("rel_pos", [N, N], BF16, kind="ExternalInput").ap(),
        "W_qkv": nc.dram_tensor("W_qkv", [E, D], F32, kind="ExternalInput").ap(),
        "b_qkv": nc.dram_tensor("b_qkv", [E], F32, kind="ExternalInput").ap(),
        "W_d": nc.dram_tensor("W_d", [D, 1], F32, kind="ExternalInput").ap(),
        "b_d": nc.dram_tensor("b_d", [D], F32, kind="ExternalInput").ap(),
        "out": nc.dram_tensor("out", [N, D], F32, kind="ExternalOutput").ap(),
    }
    with tile.TileContext(nc) as tc:
        with ExitStack() as ctx:
            build_kernel(ctx, tc, io)
    nc.compile()
    _CACHE["nc"] = nc
    _CACHE["io"] = io
    return nc, io


def kernel(x, rel_pos, W_qkv, b_qkv, W_d, b_d, **run_kwargs):
    nc, _ = _get_nc()
    x = np.ascontiguousarray(np.asarray(x, dtype=np.float32).astype(ml_dtypes.bfloat16))
    rel_bf = np.ascontiguousarray(
        np.asarray(rel_pos, dtype=np.float32).astype(ml_dtypes.bfloat16)
    )
    W_qkv = np.ascontiguousarray(np.asarray(W_qkv, dtype=np.float32))
    b_qkv = np.ascontiguousarray(np.asarray(b_qkv, dtype=np.float32))
    W_d = np.ascontiguousarray(np.asarray(W_d, dtype=np.float32))
    b_d = np.ascontiguousarray(np.asarray(b_d, dtype=np.float32))
    in_maps = [
        {
            "x": x[b],
            "rel_pos": rel_bf[b],
            "W_qkv": W_qkv,
            "b_qkv": b_qkv,
            "W_d": W_d,
            "b_d": b_d,
        }
        for b in range(B)
    ]
    r = run_bass_kernel_spmd(nc, in_maps, core_ids=list(range(B)), **run_kwargs)
    out = np.stack([r.results[b]["out"] for b in range(B)], axis=0)
    if run_kwargs:
        _CACHE["last_result"] = r
    return out
